# revision 29
# baseline (speedup 1.0000x reference)
"""Trainium2 Bass kernel for nn_CrossAttFA (retrieval_knn).

Math (reference):
  q = W @ x1 (1x1 conv, per-view), k = W @ x2, v = x3
  Q = l2norm(unfold3x3(q) regrouped to [b, L, 1800]), K likewise
  attn = Q @ K^T  [b, L, L];  idx = argmax(attn, -1)
  out = fold3x3(gather rows of unfold(v) by idx)

Device formulation (per batch b): fold the horizontal patch shift dx into
channels: qp[(a,c,dx), u] = q[a,c, uy-1, x+dx-1] over a vertically padded
50x48 pixel grid (u = uy*48+x, uy in [0,50)).  Then with
  S[u, v] = sum_ch qp[ch, u] * kp[ch, v]           (600-dim contraction)
  attn[n, m] = sum_{dy in 0..2} S[n + 48*dy, m + 48*dy]
argmax_m attn[n,m]/||K_m|| equals the reference argmax (column scale rk[m]
preserved; row scale 1/||Q_n|| does not affect argmax).

Precision scheme: the device computes attn in bf16 (inputs rounded to
bf16; matmul products exact, fp32 accumulate) and returns the TOP-8
values + indices per row (hardware max/max_index emit 8 lanes anyway).
The host rescores the <=8 candidates exactly in fp64 and picks the true
argmax; rows whose device top-8 spread is too small to certify coverage
(v1 - v8 < THETA) get a full exact row recompute.  Measured device error
is <5e-3 abs while the min top-8 spread is 4e-2, so flags are rare.

Sharding: 8 cores = 2 batches x 4 row-slabs of 576 attention rows each.
Each core computes S rows for its slab (+96 halo), the 3-term diagonal
box-sum (one add on GpSimd, one on DVE), the rk column scale, and the
hardware top-8 max/max_index.  Host does the 1x1 conv + layout prep and
the rescore + gather/fold epilogue.
"""
import sys

sys.path.insert(0, '/opt/trn_rl_repo')
import numpy as np
import ml_dtypes

B, C, AH, AW, H, W_ = 2, 64, 5, 5, 48, 48
A = AH * AW                  # 25 views
L = H * W_                   # 2304 pixels
CH = A * C // 8 * 3          # 600 channels (a, c_out=8, dx=3)
CO = 8                       # conv output channels
CHP = 640                    # padded to 5 K-chunks of 128
UR = 2400                    # padded u-grid rows (50 x 48)
NCORES = 8
SLAB = L // 4                # 576 attn rows per core
USLAB = SLAB + 96            # S rows needed per core (incl. +48,+96 halo)
NT = 480                     # matmul moving free dim (psum bank = 512 fp32)
NB = 480                     # matmul moving free dim (ISA caps at 512)
THETA = 0.022                # top-8 spread flag threshold (abs, scaled units)
KLAST = 88                   # real channels in the 5th K-chunk (600 - 512)

_PROG = None


def _build_program():
    import concourse.bass as bass
    import concourse.bacc as bacc
    import concourse.mybir as mybir
    from concourse.tile import TileContext

    nc = bacc.Bacc('TRN2', target_bir_lowering=False, debug=False,
                   num_devices=NCORES)
    qpT_in = nc.declare_dram_parameter("qpT", [5, 128, USLAB],
                                       mybir.dt.bfloat16, isOutput=False)
    kpT_in = nc.declare_dram_parameter("kpT", [5, 128, UR],
                                       mybir.dt.bfloat16, isOutput=False)
    rk_in = nc.declare_dram_parameter("rk", [128, L],
                                      mybir.dt.bfloat16, isOutput=False)
    idx_out = nc.declare_dram_parameter("idx", [128, 5, 8],
                                        mybir.dt.uint16, isOutput=True)
    val_out = nc.declare_dram_parameter("val", [128, 5, 8],
                                        mybir.dt.bfloat16, isOutput=True)

    n_sp = (USLAB + 127) // 128          # 6 S-row tiles (last is 32 rows)
    sp_rows = [min(128, USLAB - 128 * t) for t in range(n_sp)]
    n_at = (SLAB + 127) // 128           # 5 attn tiles (last is 64 rows)
    at_rows = [min(128, SLAB - 128 * t) for t in range(n_at)]

    with TileContext(nc) as tc:
        with tc.tile_pool(name="inp", bufs=1) as inp, \
             tc.tile_pool(name="sp", bufs=6) as spp, \
             tc.tile_pool(name="acc", bufs=2) as accp, \
             tc.tile_pool(name="res", bufs=2) as resp, \
             tc.tile_pool(name="ps", bufs=1, space="PSUM") as psp:

            kp_t = [inp.tile([128, UR], mybir.dt.bfloat16, tag=f"kp{i}",
                             name=f"kp{i}") for i in range(5)]
            qp_t = [inp.tile([128, USLAB], mybir.dt.bfloat16, tag=f"qp{i}",
                             name=f"qp{i}") for i in range(5)]
            rk_t = inp.tile([128, L], mybir.dt.bfloat16, tag="rk")
            # One DMA per kc chunk: more SP issues would serialize on the
            # SP sequencer (~0.7us each) and delay the staging DMAs that
            # feed the DVE.  qp/rk go on the Act queue.
            # All input issues on SP: the Act queue must stay free for the
            # psum->sbuf copies (its FIFO head otherwise delays them ~8us).
            for i in range(5):
                rows_k = KLAST if i == 4 else 128
                nc.sync.dma_start(kp_t[i][:rows_k, :], kpT_in[i][:rows_k, :])
                nc.sync.dma_start(qp_t[i][:rows_k, :], qpT_in[i][:rows_k, :])
            nc.sync.dma_start(rk_t[:], rk_in[:])
            mxc = inp.tile([128, 5, 8], mybir.dt.bfloat16, tag="mxc")
            mic = inp.tile([128, 5, 8], mybir.dt.uint16, tag="mic")
            # GpSimd ucode warm-up: its first tensor_add pays a
            # LIBRARY_RELOAD; absorb it at t=0 instead of mid-pipeline.
            warm = inp.tile([128, 8], mybir.dt.bfloat16, tag="warm")
            nc.gpsimd.memset(warm[:], 0)
            nc.gpsimd.tensor_add(warm[:], warm[:], warm[:])

            sp_tiles = [None] * n_sp

            def make_sp(t):
                # j-outer loop: each PSUM bank finishes its 5-chunk
                # accumulation then is copied out while the next bank
                # computes, so bank j is free again well before the next
                # S tile needs it (no PSUM stall with bufs=1).  S sbuf
                # copy is bf16 (halves staging DMA bytes; the rounding is
                # scaled down by rk and stays well inside the margin).
                rows = sp_rows[t]
                sp = spp.tile([128, UR], mybir.dt.bfloat16, tag="sp")
                for j, c0 in enumerate(range(0, UR, NB)):
                    w = min(NB, UR - c0)
                    ps = psp.tile([128, w], mybir.dt.float32, tag=f"ps{j}",
                                  name=f"ps{j}")
                    for kc in range(5):
                        kk = KLAST if kc == 4 else 128
                        nc.tensor.matmul(
                            ps[:rows, :],
                            qp_t[kc][:kk, 128 * t:128 * t + rows],
                            kp_t[kc][:kk, c0:c0 + w],
                            start=(kc == 0), stop=(kc == 4))
                    nc.scalar.copy(sp[:rows, c0:c0 + w], ps[:rows, :])
                sp_tiles[t] = sp

            def attn_tile(t):
                rows = at_rows[t]
                a0 = 128 * t  # slab-local first attn row of this tile
                # term dy contributes S[a0+r+48dy, m+48dy]; S tile k holds
                # rows [128k, 128k + sp_rows[k]).
                def pieces(dy):
                    out = []
                    lo = a0 + 48 * dy
                    hi = lo + rows
                    k = lo // 128
                    while lo < hi:
                        stop = min(hi, 128 * (k + 1))
                        out.append((k, lo - 128 * k, lo - a0 - 48 * dy,
                                    stop - lo))
                        lo = stop
                        k += 1
                    return out
                # DVE requires equal base partitions for SBUF operands, so
                # the +48/+96 partition-phase terms are staged through DMA
                # (which has no partition constraints), then added aligned.
                # Whole post-matmul chain in bf16: all-2-byte operands give
                # the DVE ADD/MULT their 2x mode, and the rounding errors
                # are damped by the rk scale (~0.07), staying below THETA.
                # The s48+s96 add runs on the DVE for the first two tiles
                # (shorter critical prefix) and on GpSimd afterwards
                # (offload, DVE saturated by then).
                s48 = accp.tile([128, L], mybir.dt.bfloat16, tag="s48")
                s96 = accp.tile([128, L], mybir.dt.bfloat16, tag="s96")
                acc = accp.tile([128, L], mybir.dt.bfloat16, tag="acc")
                accs = accp.tile([128, L], mybir.dt.bfloat16, tag="accs")
                for (k, srow, arow, n) in pieces(1):
                    nc.sync.dma_start(
                        s48[arow:arow + n, :],
                        sp_tiles[k][srow:srow + n, 48:48 + L])
                for (k, srow, arow, n) in pieces(2):
                    nc.sync.dma_start(
                        s96[arow:arow + n, :],
                        sp_tiles[k][srow:srow + n, 96:96 + L])
                # First and last tiles: adds on the DVE (shortest chain at
                # the pipeline ends); middle tiles: the s48+s96 add goes to
                # GpSimd (offload) but the +term0 add stays on the DVE —
                # two serial GpSimd passes per tile would outpace the DVE
                # and become the mid-pipeline limiter.
                eng = nc.vector if t in (0, n_at - 1) else nc.gpsimd
                eng.tensor_add(acc[:rows, :], s48[:rows, :], s96[:rows, :])
                nc.vector.tensor_add(acc[:rows, :], acc[:rows, :],
                                     sp_tiles[t][:rows, 0:L])
                eng.tensor_mul(accs[:rows, :], acc[:rows, :],
                               rk_t[:rows, :])
                nc.vector.max(mxc[:rows, t, :], accs[:rows, :])
                nc.vector.max_index(mic[:rows, t, :], mxc[:rows, t, :],
                                    accs[:rows, :])

            make_sp(0)
            for t in range(1, n_sp):
                make_sp(t)
                attn_tile(t - 1)
            nc.sync.dma_start(val_out[:], mxc[:])
            nc.sync.dma_start(idx_out[:], mic[:])

    nc.compile()
    return nc


def _host_prep(x1, x2, w):
    """Build qp/kp [B,600,UR] fp32, their padded bf16 device forms,
    rk fp32 [B,L] and rk64 [B,L]."""
    x1f = x1.transpose(0, 2, 3, 1, 4, 5).reshape(B, A, C, H, W_)
    x2f = x2.transpose(0, 2, 3, 1, 4, 5).reshape(B, A, C, H, W_)
    q = np.einsum('oc,bachw->baohw', w, x1f)   # [B, A, 8, H, W]
    k = np.einsum('oc,bachw->baohw', w, x2f)

    def chanshift(g):
        # g [B, A, 8, H, W] -> [B, 600, 50*48] with (a, c, dx) channels on a
        # vertically padded 50x48 grid
        gp = np.pad(g, ((0, 0), (0, 0), (0, 0), (0, 0), (1, 1)))
        sh = np.stack([gp[..., dx:dx + W_] for dx in range(3)], axis=3)
        sh = sh.reshape(B, CH, H, W_)
        sh = np.pad(sh, ((0, 0), (0, 0), (1, 1), (0, 0)))
        return np.ascontiguousarray(sh.reshape(B, CH, UR), dtype=np.float32)

    qp = chanshift(q)
    kp = chanshift(k)
    # rk[m] = 1 / ||K_m||, from padded per-pixel energy box-sums (fp64)
    ek = (k.astype(np.float64) ** 2).sum(axis=(1, 2))        # [B, H, W]
    ekp = np.pad(ek, ((0, 0), (1, 1), (1, 1)))
    kn = sum(ekp[:, dy:dy + H, dx:dx + W_]
             for dy in range(3) for dx in range(3))
    rk64 = (1.0 / np.maximum(np.sqrt(kn), 1e-12)).reshape(B, L)

    def to_dev(g):
        gb = g.astype(ml_dtypes.bfloat16)
        pad = np.zeros((B, CHP - CH, UR), ml_dtypes.bfloat16)
        return np.concatenate([gb, pad], axis=1).reshape(B, 5, 128, UR)

    return (qp, kp, to_dev(qp), to_dev(kp),
            rk64.astype(ml_dtypes.bfloat16), rk64)


def _resolve_idx(qp, kp, rk64, top8, vals):
    """Pick the exact (fp64) argmax among device top-8 candidates; rows
    with uncertifiably small top-8 spread get a full-row recompute."""
    idx = np.zeros((B, L), np.int64)
    for b in range(B):
        cand = top8[b].astype(np.int64)          # [L, 8]
        q64 = qp[b].astype(np.float64)           # [600, UR]
        k64 = kp[b].astype(np.float64)
        score = np.zeros((L, 8))
        for dy in (0, 48, 96):
            Qd = q64[:, dy:dy + L]               # [600, L]
            for c0 in range(0, L, 384):
                sl = slice(c0, c0 + 384)
                Kd = k64[:, cand[sl] + dy]       # [600, chunk, 8]
                score[sl] += np.einsum('cr,crk->rk', Qd[:, sl], Kd)
        score *= rk64[b][cand]
        pick = np.argmax(score, axis=1)
        idx[b] = cand[np.arange(L), pick]

        flagged = np.where(vals[b][:, 0] - vals[b][:, 7] < THETA)[0]
        if len(flagged):
            Qr = np.stack([q64[:, flagged + dy] for dy in (0, 48, 96)])
            Sr = np.einsum('dcr,cv->drv', Qr, k64)   # [3, R, UR]
            accs = (Sr[0][:, 0:L] + Sr[1][:, 48:48 + L]
                    + Sr[2][:, 96:96 + L]) * rk64[b][None, :]
            idx[b][flagged] = np.argmax(accs, axis=1)
    return idx


def _gather_fold(x3, idx):
    """Host epilogue: gather unfold(v) rows by idx and fold back."""
    v = x3.transpose(0, 2, 3, 1, 4, 5).reshape(B * A, C, H, W_)
    vp = np.pad(v, ((0, 0), (0, 0), (1, 1), (1, 1)))
    cols = np.stack([vp[:, :, i:i + H, j:j + W_]
                     for i in range(3) for j in range(3)], axis=2)
    V = cols.reshape(B, A, C * 9, L).transpose(0, 3, 1, 2).reshape(B, L, -1)
    outc = np.take_along_axis(V, idx[:, :, None], axis=1)
    p_v = C * 9
    outc = outc.reshape(B, L, A, p_v).transpose(0, 2, 3, 1)
    outc = outc.reshape(B * A, C, 3, 3, H, W_)
    out = np.zeros((B * A, C, H + 2, W_ + 2), np.float32)
    for i in range(3):
        for j in range(3):
            out[:, :, i:i + H, j:j + W_] += outc[:, :, i, j]
    out = out[:, :, 1:1 + H, 1:1 + W_]
    return np.ascontiguousarray(
        out.reshape(B, AH, AW, C, H, W_).transpose(0, 3, 1, 2, 4, 5))


def _make_in_maps(qpb, kpb, rk):
    in_maps = []
    for core in range(NCORES):
        b, r = core // 4, core % 4
        u0 = SLAB * r
        in_maps.append({
            "qpT": np.ascontiguousarray(qpb[b][:, :, u0:u0 + USLAB]),
            "kpT": kpb[b],
            "rk": np.broadcast_to(rk[b], (128, L)).copy(),
        })
    return in_maps


def kernel(x1, x2, x3, W):
    global _PROG
    sys.path.insert(0, '/opt/trn_rl_repo')
    from concourse.bass_utils import run_bass_kernel_spmd

    x1 = np.asarray(x1, dtype=np.float32)
    x2 = np.asarray(x2, dtype=np.float32)
    x3 = np.asarray(x3, dtype=np.float32)
    w = np.asarray(W, dtype=np.float32)

    qp, kp, qpb, kpb, rk, rk64 = _host_prep(x1, x2, w)
    in_maps = _make_in_maps(qpb, kpb, rk)

    if _PROG is None:
        _PROG = _build_program()
    res = run_bass_kernel_spmd(_PROG, in_maps, list(range(NCORES)))

    top8 = np.zeros((B, L, 8), np.int64)
    vals = np.zeros((B, L, 8), np.float32)
    for core in range(NCORES):
        b, r = core // 4, core % 4
        sl = slice(SLAB * r, SLAB * (r + 1))
        top8[b][sl] = res.results[core]["idx"].transpose(1, 0, 2).reshape(
            5 * 128, 8)[:SLAB]
        vals[b][sl] = res.results[core]["val"].transpose(1, 0, 2).reshape(
            5 * 128, 8)[:SLAB].astype(np.float32)

    idx = _resolve_idx(qp, kp, rk64, top8, vals)
    return _gather_fold(x3, idx)


# revision 35
# speedup vs baseline: 1.0845x; 1.0845x over previous
"""Trainium2 Bass kernel for nn_CrossAttFA (retrieval_knn).

Math (reference):
  q = W @ x1 (1x1 conv, per-view), k = W @ x2, v = x3
  Q = l2norm(unfold3x3(q) regrouped to [b, L, 1800]), K likewise
  attn = Q @ K^T  [b, L, L];  idx = argmax(attn, -1)
  out = fold3x3(gather rows of unfold(v) by idx)

Device formulation (per batch b): fold the horizontal patch shift dx into
channels: qp[(a,c,dx), u] = q[a,c, uy-1, x+dx-1] over a vertically padded
50x48 pixel grid (u = uy*48+x, uy in [0,50)).  Then with
  S[u, v] = sum_ch qp[ch, u] * kp[ch, v]           (600-dim contraction)
  attn[n, m] = sum_{dy in 0..2} S[n + 48*dy, m + 48*dy]
argmax_m attn[n,m]/||K_m|| equals the reference argmax (column scale rk[m]
preserved; row scale 1/||Q_n|| does not affect argmax).

Precision scheme: the device computes attn in bf16 (inputs rounded to
bf16; matmul products exact, fp32 accumulate) and returns the TOP-8
values + indices per row (hardware max/max_index emit 8 lanes anyway).
The host rescores the <=8 candidates exactly in fp64 and picks the true
argmax; rows whose device top-8 spread is too small to certify coverage
(v1 - v8 < THETA) get a full exact row recompute.  Measured device error
is <5e-3 abs while the min top-8 spread is 4e-2, so flags are rare.

Sharding: 8 cores = 2 batches x 4 row-slabs of 576 attention rows each.
Each core computes S rows for its slab (+96 halo), the 3-term diagonal
box-sum (one add on GpSimd, one on DVE), the rk column scale, and the
hardware top-8 max/max_index.  Host does the 1x1 conv + layout prep and
the rescore + gather/fold epilogue.
"""
import sys

sys.path.insert(0, '/opt/trn_rl_repo')
import numpy as np
import ml_dtypes

B, C, AH, AW, H, W_ = 2, 64, 5, 5, 48, 48
A = AH * AW                  # 25 views
L = H * W_                   # 2304 pixels
CH = A * C // 8 * 3          # 600 channels (a, c_out=8, dx=3)
CO = 8                       # conv output channels
CHP = 640                    # padded to 5 K-chunks of 128
UR = 2400                    # padded u-grid rows (50 x 48)
NCORES = 8
SLAB = L // 4                # 576 attn rows per core
USLAB = SLAB + 96            # S rows needed per core (incl. +48,+96 halo)
NT = 480                     # matmul moving free dim (psum bank = 512 fp32)
NB = 480                     # matmul moving free dim (ISA caps at 512)
THETA = 0.022                # top-8 spread flag threshold (abs, scaled units)
KLAST = 88                   # real channels in the 5th K-chunk (600 - 512)

_PROG = None


def _build_program():
    import concourse.bass as bass
    import concourse.bacc as bacc
    import concourse.mybir as mybir
    from concourse.tile import TileContext

    nc = bacc.Bacc('TRN2', target_bir_lowering=False, debug=False,
                   num_devices=NCORES)
    qpT_in = nc.declare_dram_parameter("qpT", [5, 128, USLAB],
                                       mybir.dt.bfloat16, isOutput=False)
    kpT_in = nc.declare_dram_parameter("kpT", [5, 128, UR],
                                       mybir.dt.bfloat16, isOutput=False)
    rk_in = nc.declare_dram_parameter("rk", [128, L],
                                      mybir.dt.bfloat16, isOutput=False)
    idx_out = nc.declare_dram_parameter("idx", [128, 5, 8],
                                        mybir.dt.uint16, isOutput=True)
    val_out = nc.declare_dram_parameter("val", [128, 5, 8],
                                        mybir.dt.bfloat16, isOutput=True)

    n_sp = (USLAB + 127) // 128          # 6 S-row tiles (last is 32 rows)
    sp_rows = [min(128, USLAB - 128 * t) for t in range(n_sp)]
    n_at = (SLAB + 127) // 128           # 5 attn tiles (last is 64 rows)
    at_rows = [min(128, SLAB - 128 * t) for t in range(n_at)]

    with TileContext(nc) as tc:
        with tc.tile_pool(name="inp", bufs=1) as inp, \
             tc.tile_pool(name="sp", bufs=6) as spp, \
             tc.tile_pool(name="acc", bufs=2) as accp, \
             tc.tile_pool(name="res", bufs=2) as resp, \
             tc.tile_pool(name="ps", bufs=1, space="PSUM") as psp:

            kp_t = [inp.tile([128, UR], mybir.dt.bfloat16, tag=f"kp{i}",
                             name=f"kp{i}") for i in range(5)]
            qp_t = [inp.tile([128, USLAB], mybir.dt.bfloat16, tag=f"qp{i}",
                             name=f"qp{i}") for i in range(5)]
            rk_t = inp.tile([128, L], mybir.dt.bfloat16, tag="rk")
            rk2_t = inp.tile([128, L // 2], mybir.dt.bfloat16, tag="rk2")
            # One DMA per kc chunk: more SP issues would serialize on the
            # SP sequencer (~0.7us each) and delay the staging DMAs that
            # feed the DVE.  qp/rk go on the Act queue.
            # All input issues on SP: the Act queue must stay free for the
            # psum->sbuf copies (its FIFO head otherwise delays them ~8us).
            for i in range(5):
                rows_k = KLAST if i == 4 else 128
                nc.sync.dma_start(kp_t[i][:rows_k, :], kpT_in[i][:rows_k, :])
                nc.sync.dma_start(qp_t[i][:rows_k, :], qpT_in[i][:rows_k, :])
            nc.sync.dma_start(rk_t[:], rk_in[:])
            # rk2: per-half rk for the repacked last attn tile
            nc.sync.dma_start(rk2_t[0:64, :], rk_t[0:64, 0:L // 2])
            nc.sync.dma_start(rk2_t[64:128, :], rk_t[0:64, L // 2:L])
            mxc = inp.tile([128, 5, 8], mybir.dt.bfloat16, tag="mxc")
            mic = inp.tile([128, 5, 8], mybir.dt.uint16, tag="mic")
            # GpSimd ucode warm-up: its first tensor_add pays a
            # LIBRARY_RELOAD; absorb it at t=0 instead of mid-pipeline.
            warm = inp.tile([128, 8], mybir.dt.bfloat16, tag="warm")
            nc.gpsimd.memset(warm[:], 0)
            nc.gpsimd.tensor_add(warm[:], warm[:], warm[:])

            sp_tiles = [None] * n_sp

            def make_sp(t):
                # j-outer loop: each PSUM bank finishes its 5-chunk
                # accumulation then is copied out while the next bank
                # computes, so bank j is free again well before the next
                # S tile needs it (no PSUM stall with bufs=1).  S sbuf
                # copy is bf16 (halves staging DMA bytes; the rounding is
                # scaled down by rk and stays well inside the margin).
                rows = sp_rows[t]
                sp = spp.tile([128, UR], mybir.dt.bfloat16, tag="sp")
                for j, c0 in enumerate(range(0, UR, NB)):
                    w = min(NB, UR - c0)
                    ps = psp.tile([128, w], mybir.dt.float32, tag=f"ps{j}",
                                  name=f"ps{j}")
                    for kc in range(5):
                        kk = KLAST if kc == 4 else 128
                        nc.tensor.matmul(
                            ps[:rows, :],
                            qp_t[kc][:kk, 128 * t:128 * t + rows],
                            kp_t[kc][:kk, c0:c0 + w],
                            start=(kc == 0), stop=(kc == 4))
                    nc.scalar.copy(sp[:rows, c0:c0 + w], ps[:rows, :])
                sp_tiles[t] = sp

            def attn_tile(t):
                rows = at_rows[t]
                a0 = 128 * t  # slab-local first attn row of this tile
                # term dy contributes S[a0+r+48dy, m+48dy]; S tile k holds
                # rows [128k, 128k + sp_rows[k]).
                def pieces(dy):
                    out = []
                    lo = a0 + 48 * dy
                    hi = lo + rows
                    k = lo // 128
                    while lo < hi:
                        stop = min(hi, 128 * (k + 1))
                        out.append((k, lo - 128 * k, lo - a0 - 48 * dy,
                                    stop - lo))
                        lo = stop
                        k += 1
                    return out
                # DVE requires equal base partitions for SBUF operands, so
                # the +48/+96 partition-phase terms are staged through DMA
                # (which has no partition constraints), then added aligned.
                # Whole post-matmul chain in bf16: all-2-byte operands give
                # the DVE ADD/MULT their 2x mode, and the rounding errors
                # are damped by the rk scale (~0.07), staying below THETA.
                # The s48+s96 add runs on the DVE for the first two tiles
                # (shorter critical prefix) and on GpSimd afterwards
                # (offload, DVE saturated by then).
                s48 = accp.tile([128, L], mybir.dt.bfloat16, tag="s48")
                s96 = accp.tile([128, L], mybir.dt.bfloat16, tag="s96")
                acc = accp.tile([128, L], mybir.dt.bfloat16, tag="acc")
                accs = accp.tile([128, L], mybir.dt.bfloat16, tag="accs")
                for (k, srow, arow, n) in pieces(1):
                    nc.sync.dma_start(
                        s48[arow:arow + n, :],
                        sp_tiles[k][srow:srow + n, 48:48 + L])
                for (k, srow, arow, n) in pieces(2):
                    nc.sync.dma_start(
                        s96[arow:arow + n, :],
                        sp_tiles[k][srow:srow + n, 96:96 + L])
                # First and last tiles: adds on the DVE (shortest chain at
                # the pipeline ends); middle tiles: the s48+s96 add goes to
                # GpSimd (offload) but the +term0 add stays on the DVE —
                # two serial GpSimd passes per tile would outpace the DVE
                # and become the mid-pipeline limiter.
                eng = nc.vector if t == 0 else nc.gpsimd
                eng.tensor_add(acc[:rows, :], s48[:rows, :], s96[:rows, :])
                nc.vector.tensor_add(acc[:rows, :], acc[:rows, :],
                                     sp_tiles[t][:rows, 0:L])
                nc.vector.tensor_mul(accs[:rows, :], acc[:rows, :],
                                     rk_t[:rows, :])
                nc.vector.max(mxc[:rows, t, :], accs[:rows, :])
                nc.vector.max_index(mic[:rows, t, :], mxc[:rows, t, :],
                                    accs[:rows, :])

            def attn_tile_last():
                # The ragged 64-row last tile is repacked as [128, L/2]:
                # partitions 0:64 hold columns [0, L/2), partitions 64:128
                # hold columns [L/2, L) — every DVE pass runs at half width.
                # The staging DMAs do the partition/column remap for free;
                # the host merges the two 8-candidate halves.
                HW = L // 2
                t0h = accp.tile([128, HW], mybir.dt.bfloat16, tag="t0h")
                s48h = accp.tile([128, HW], mybir.dt.bfloat16, tag="s48h")
                s96h = accp.tile([128, HW], mybir.dt.bfloat16, tag="s96h")
                acch = accp.tile([128, HW], mybir.dt.bfloat16, tag="acch")
                accsh = accp.tile([128, HW], mybir.dt.bfloat16, tag="accsh")
                nc.sync.dma_start(t0h[0:64, :], sp_tiles[4][0:64, 0:HW])
                nc.sync.dma_start(t0h[64:128, :], sp_tiles[4][0:64, HW:L])
                nc.sync.dma_start(s48h[0:64, :],
                                  sp_tiles[4][48:112, 48:48 + HW])
                nc.sync.dma_start(s48h[64:128, :],
                                  sp_tiles[4][48:112, 48 + HW:48 + L])
                nc.sync.dma_start(s96h[0:32, :],
                                  sp_tiles[4][96:128, 96:96 + HW])
                nc.sync.dma_start(s96h[64:96, :],
                                  sp_tiles[4][96:128, 96 + HW:96 + L])
                nc.sync.dma_start(s96h[32:64, :],
                                  sp_tiles[5][0:32, 96:96 + HW])
                nc.sync.dma_start(s96h[96:128, :],
                                  sp_tiles[5][0:32, 96 + HW:96 + L])
                nc.vector.tensor_add(acch[:], t0h[:], s48h[:])
                nc.vector.tensor_add(acch[:], acch[:], s96h[:])
                nc.vector.tensor_mul(accsh[:], acch[:], rk2_t[:])
                nc.vector.max(mxc[:, 4, :], accsh[:])
                nc.vector.max_index(mic[:, 4, :], mxc[:, 4, :], accsh[:])

            make_sp(0)
            for t in range(1, n_sp):
                make_sp(t)
                if t - 1 < n_at - 1:
                    attn_tile(t - 1)
            attn_tile_last()
            nc.sync.dma_start(val_out[:], mxc[:])
            nc.sync.dma_start(idx_out[:], mic[:])

    nc.compile()
    return nc


def _host_prep(x1, x2, w):
    """Build qp/kp [B,600,UR] fp32, their padded bf16 device forms,
    rk fp32 [B,L] and rk64 [B,L]."""
    x1f = x1.transpose(0, 2, 3, 1, 4, 5).reshape(B, A, C, H, W_)
    x2f = x2.transpose(0, 2, 3, 1, 4, 5).reshape(B, A, C, H, W_)
    q = np.einsum('oc,bachw->baohw', w, x1f)   # [B, A, 8, H, W]
    k = np.einsum('oc,bachw->baohw', w, x2f)

    def chanshift(g):
        # g [B, A, 8, H, W] -> [B, 600, 50*48] with (a, c, dx) channels on a
        # vertically padded 50x48 grid
        gp = np.pad(g, ((0, 0), (0, 0), (0, 0), (0, 0), (1, 1)))
        sh = np.stack([gp[..., dx:dx + W_] for dx in range(3)], axis=3)
        sh = sh.reshape(B, CH, H, W_)
        sh = np.pad(sh, ((0, 0), (0, 0), (1, 1), (0, 0)))
        return np.ascontiguousarray(sh.reshape(B, CH, UR), dtype=np.float32)

    qp = chanshift(q)
    kp = chanshift(k)
    # rk[m] = 1 / ||K_m||, from padded per-pixel energy box-sums (fp64)
    ek = (k.astype(np.float64) ** 2).sum(axis=(1, 2))        # [B, H, W]
    ekp = np.pad(ek, ((0, 0), (1, 1), (1, 1)))
    kn = sum(ekp[:, dy:dy + H, dx:dx + W_]
             for dy in range(3) for dx in range(3))
    rk64 = (1.0 / np.maximum(np.sqrt(kn), 1e-12)).reshape(B, L)

    def to_dev(g):
        gb = g.astype(ml_dtypes.bfloat16)
        pad = np.zeros((B, CHP - CH, UR), ml_dtypes.bfloat16)
        return np.concatenate([gb, pad], axis=1).reshape(B, 5, 128, UR)

    return (qp, kp, to_dev(qp), to_dev(kp),
            rk64.astype(ml_dtypes.bfloat16), rk64)


def _resolve_idx(qp, kp, rk64, top8, flags):
    """Pick the exact (fp64) argmax among device candidates; rows with
    uncertifiably small top-8 spread get a full-row recompute."""
    nc_ = top8.shape[-1]
    idx = np.zeros((B, L), np.int64)
    for b in range(B):
        cand = top8[b].astype(np.int64)          # [L, nc_]
        q64 = qp[b].astype(np.float64)           # [600, UR]
        k64 = kp[b].astype(np.float64)
        score = np.zeros((L, nc_))
        for dy in (0, 48, 96):
            Qd = q64[:, dy:dy + L]               # [600, L]
            for c0 in range(0, L, 384):
                sl = slice(c0, c0 + 384)
                Kd = k64[:, cand[sl] + dy]       # [600, chunk, nc_]
                score[sl] += np.einsum('cr,crk->rk', Qd[:, sl], Kd)
        score *= rk64[b][cand]
        pick = np.argmax(score, axis=1)
        idx[b] = cand[np.arange(L), pick]

        flagged = np.where(flags[b])[0]
        if len(flagged):
            Qr = np.stack([q64[:, flagged + dy] for dy in (0, 48, 96)])
            Sr = np.einsum('dcr,cv->drv', Qr, k64)   # [3, R, UR]
            accs = (Sr[0][:, 0:L] + Sr[1][:, 48:48 + L]
                    + Sr[2][:, 96:96 + L]) * rk64[b][None, :]
            idx[b][flagged] = np.argmax(accs, axis=1)
    return idx


def _gather_fold(x3, idx):
    """Host epilogue: gather unfold(v) rows by idx and fold back."""
    v = x3.transpose(0, 2, 3, 1, 4, 5).reshape(B * A, C, H, W_)
    vp = np.pad(v, ((0, 0), (0, 0), (1, 1), (1, 1)))
    cols = np.stack([vp[:, :, i:i + H, j:j + W_]
                     for i in range(3) for j in range(3)], axis=2)
    V = cols.reshape(B, A, C * 9, L).transpose(0, 3, 1, 2).reshape(B, L, -1)
    outc = np.take_along_axis(V, idx[:, :, None], axis=1)
    p_v = C * 9
    outc = outc.reshape(B, L, A, p_v).transpose(0, 2, 3, 1)
    outc = outc.reshape(B * A, C, 3, 3, H, W_)
    out = np.zeros((B * A, C, H + 2, W_ + 2), np.float32)
    for i in range(3):
        for j in range(3):
            out[:, :, i:i + H, j:j + W_] += outc[:, :, i, j]
    out = out[:, :, 1:1 + H, 1:1 + W_]
    return np.ascontiguousarray(
        out.reshape(B, AH, AW, C, H, W_).transpose(0, 3, 1, 2, 4, 5))


def _make_in_maps(qpb, kpb, rk):
    in_maps = []
    for core in range(NCORES):
        b, r = core // 4, core % 4
        u0 = SLAB * r
        in_maps.append({
            "qpT": np.ascontiguousarray(qpb[b][:, :, u0:u0 + USLAB]),
            "kpT": kpb[b],
            "rk": np.broadcast_to(rk[b], (128, L)).copy(),
        })
    return in_maps


def kernel(x1, x2, x3, W):
    global _PROG
    sys.path.insert(0, '/opt/trn_rl_repo')
    from concourse.bass_utils import run_bass_kernel_spmd

    x1 = np.asarray(x1, dtype=np.float32)
    x2 = np.asarray(x2, dtype=np.float32)
    x3 = np.asarray(x3, dtype=np.float32)
    w = np.asarray(W, dtype=np.float32)

    qp, kp, qpb, kpb, rk, rk64 = _host_prep(x1, x2, w)
    in_maps = _make_in_maps(qpb, kpb, rk)

    if _PROG is None:
        _PROG = _build_program()
    res = run_bass_kernel_spmd(_PROG, in_maps, list(range(NCORES)))

    top8 = np.zeros((B, L, 16), np.int64)
    flags = np.zeros((B, L), bool)
    for core in range(NCORES):
        b, r = core // 4, core % 4
        base = SLAB * r
        ri = res.results[core]["idx"].astype(np.int64)      # [128, 5, 8]
        rv = res.results[core]["val"].astype(np.float32)
        mi = ri[:, 0:4, :].transpose(1, 0, 2).reshape(512, 8)
        mv = rv[:, 0:4, :].transpose(1, 0, 2).reshape(512, 8)
        top8[b, base:base + 512, 0:8] = mi
        top8[b, base:base + 512, 8:16] = mi
        flags[b, base:base + 512] = (mv[:, 0] - mv[:, 7]) < THETA
        # repacked last tile: two column halves per row
        h0i, h1i = ri[0:64, 4, :], ri[64:128, 4, :] + L // 2
        h0v, h1v = rv[0:64, 4, :], rv[64:128, 4, :]
        top8[b, base + 512:base + SLAB, 0:8] = h0i
        top8[b, base + 512:base + SLAB, 8:16] = h1i
        flags[b, base + 512:base + SLAB] = (
            (h0v[:, 0] - h0v[:, 7] < THETA)
            | (h1v[:, 0] - h1v[:, 7] < THETA))

    idx = _resolve_idx(qp, kp, rk64, top8, flags)
    return _gather_fold(x3, idx)


# revision 36
# speedup vs baseline: 1.0898x; 1.0049x over previous
"""Trainium2 Bass kernel for nn_CrossAttFA (retrieval_knn).

Math (reference):
  q = W @ x1 (1x1 conv, per-view), k = W @ x2, v = x3
  Q = l2norm(unfold3x3(q) regrouped to [b, L, 1800]), K likewise
  attn = Q @ K^T  [b, L, L];  idx = argmax(attn, -1)
  out = fold3x3(gather rows of unfold(v) by idx)

Device formulation (per batch b): fold the horizontal patch shift dx into
channels: qp[(a,c,dx), u] = q[a,c, uy-1, x+dx-1] over a vertically padded
50x48 pixel grid (u = uy*48+x, uy in [0,50)).  Then with
  S[u, v] = sum_ch qp[ch, u] * kp[ch, v]           (600-dim contraction)
  attn[n, m] = sum_{dy in 0..2} S[n + 48*dy, m + 48*dy]
argmax_m attn[n,m]/||K_m|| equals the reference argmax (column scale rk[m]
preserved; row scale 1/||Q_n|| does not affect argmax).

Precision scheme: the device computes attn in bf16 (inputs rounded to
bf16; matmul products exact, fp32 accumulate) and returns the TOP-8
values + indices per row (hardware max/max_index emit 8 lanes anyway).
The host rescores the <=8 candidates exactly in fp64 and picks the true
argmax; rows whose device top-8 spread is too small to certify coverage
(v1 - v8 < THETA) get a full exact row recompute.  Measured device error
is <5e-3 abs while the min top-8 spread is 4e-2, so flags are rare.

Sharding: 8 cores = 2 batches x 4 row-slabs of 576 attention rows each.
Each core computes S rows for its slab (+96 halo), the 3-term diagonal
box-sum (one add on GpSimd, one on DVE), the rk column scale, and the
hardware top-8 max/max_index.  Host does the 1x1 conv + layout prep and
the rescore + gather/fold epilogue.
"""
import sys

sys.path.insert(0, '/opt/trn_rl_repo')
import numpy as np
import ml_dtypes

B, C, AH, AW, H, W_ = 2, 64, 5, 5, 48, 48
A = AH * AW                  # 25 views
L = H * W_                   # 2304 pixels
CH = A * C // 8 * 3          # 600 channels (a, c_out=8, dx=3)
CO = 8                       # conv output channels
CHP = 640                    # padded to 5 K-chunks of 128
UR = 2400                    # padded u-grid rows (50 x 48)
NCORES = 8
SLAB = L // 4                # 576 attn rows per core
USLAB = SLAB + 96            # S rows needed per core (incl. +48,+96 halo)
NT = 480                     # matmul moving free dim (psum bank = 512 fp32)
NB = 480                     # matmul moving free dim (ISA caps at 512)
THETA = 0.022                # top-8 spread flag threshold (abs, scaled units)
KLAST = 88                   # real channels in the 5th K-chunk (600 - 512)

_PROG = None


def _build_program():
    import concourse.bass as bass
    import concourse.bacc as bacc
    import concourse.mybir as mybir
    from concourse.tile import TileContext

    nc = bacc.Bacc('TRN2', target_bir_lowering=False, debug=False,
                   num_devices=NCORES)
    qpT_in = nc.declare_dram_parameter("qpT", [5, 128, USLAB],
                                       mybir.dt.bfloat16, isOutput=False)
    kpT_in = nc.declare_dram_parameter("kpT", [5, 128, UR],
                                       mybir.dt.bfloat16, isOutput=False)
    rk_in = nc.declare_dram_parameter("rk", [128, L],
                                      mybir.dt.bfloat16, isOutput=False)
    idx_out = nc.declare_dram_parameter("idx", [128, 5, 8],
                                        mybir.dt.uint16, isOutput=True)
    val_out = nc.declare_dram_parameter("val", [128, 5, 8],
                                        mybir.dt.bfloat16, isOutput=True)

    n_sp = (USLAB + 127) // 128          # 6 S-row tiles (last is 32 rows)
    sp_rows = [min(128, USLAB - 128 * t) for t in range(n_sp)]
    n_at = (SLAB + 127) // 128           # 5 attn tiles (last is 64 rows)
    at_rows = [min(128, SLAB - 128 * t) for t in range(n_at)]

    with TileContext(nc) as tc:
        with tc.tile_pool(name="inp", bufs=1) as inp, \
             tc.tile_pool(name="sp", bufs=6) as spp, \
             tc.tile_pool(name="acc", bufs=2) as accp, \
             tc.tile_pool(name="res", bufs=2) as resp, \
             tc.tile_pool(name="ps", bufs=1, space="PSUM") as psp:

            kp_t = [inp.tile([128, UR], mybir.dt.bfloat16, tag=f"kp{i}",
                             name=f"kp{i}") for i in range(5)]
            qp_t = [inp.tile([128, USLAB], mybir.dt.bfloat16, tag=f"qp{i}",
                             name=f"qp{i}") for i in range(5)]
            rk_t = inp.tile([128, L], mybir.dt.bfloat16, tag="rk")
            rk2_t = inp.tile([128, L // 2], mybir.dt.bfloat16, tag="rk2")
            # One DMA per kc chunk: more SP issues would serialize on the
            # SP sequencer (~0.7us each) and delay the staging DMAs that
            # feed the DVE.  qp/rk go on the Act queue.
            # All input issues on SP: the Act queue must stay free for the
            # psum->sbuf copies (its FIFO head otherwise delays them ~8us).
            for i in range(5):
                rows_k = KLAST if i == 4 else 128
                nc.sync.dma_start(kp_t[i][:rows_k, :], kpT_in[i][:rows_k, :])
                nc.sync.dma_start(qp_t[i][:rows_k, :], qpT_in[i][:rows_k, :])
            nc.sync.dma_start(rk_t[:], rk_in[:])
            # rk2: per-half rk for the repacked last attn tile
            nc.sync.dma_start(rk2_t[0:64, :], rk_t[0:64, 0:L // 2])
            nc.sync.dma_start(rk2_t[64:128, :], rk_t[0:64, L // 2:L])
            mxc = inp.tile([128, 5, 8], mybir.dt.bfloat16, tag="mxc")
            mic = inp.tile([128, 5, 8], mybir.dt.uint16, tag="mic")
            # GpSimd ucode warm-up: its first tensor_add pays a
            # LIBRARY_RELOAD; absorb it at t=0 instead of mid-pipeline.
            warm = inp.tile([128, 8], mybir.dt.bfloat16, tag="warm")
            nc.gpsimd.memset(warm[:], 0)
            nc.gpsimd.tensor_add(warm[:], warm[:], warm[:])

            sp_tiles = [None] * n_sp

            def make_sp(t):
                # j-outer loop: each PSUM bank finishes its 5-chunk
                # accumulation then is copied out while the next bank
                # computes, so bank j is free again well before the next
                # S tile needs it (no PSUM stall with bufs=1).  S sbuf
                # copy is bf16 (halves staging DMA bytes; the rounding is
                # scaled down by rk and stays well inside the margin).
                rows = sp_rows[t]
                sp = spp.tile([128, UR], mybir.dt.bfloat16, tag="sp")
                for j, c0 in enumerate(range(0, UR, NB)):
                    w = min(NB, UR - c0)
                    ps = psp.tile([128, w], mybir.dt.float32, tag=f"ps{j}",
                                  name=f"ps{j}")
                    for kc in range(5):
                        kk = KLAST if kc == 4 else 128
                        nc.tensor.matmul(
                            ps[:rows, :],
                            qp_t[kc][:kk, 128 * t:128 * t + rows],
                            kp_t[kc][:kk, c0:c0 + w],
                            start=(kc == 0), stop=(kc == 4))
                    # Copies of the first two S tiles run on the DVE (idle
                    # until attn(0) anyway) — the Act copy stream otherwise
                    # starts ~8us late and gates the whole attn pipeline.
                    if t < 2:
                        nc.vector.tensor_copy(sp[:rows, c0:c0 + w],
                                              ps[:rows, :])
                    else:
                        nc.scalar.copy(sp[:rows, c0:c0 + w], ps[:rows, :])
                sp_tiles[t] = sp

            def attn_tile(t):
                rows = at_rows[t]
                a0 = 128 * t  # slab-local first attn row of this tile
                # term dy contributes S[a0+r+48dy, m+48dy]; S tile k holds
                # rows [128k, 128k + sp_rows[k]).
                def pieces(dy):
                    out = []
                    lo = a0 + 48 * dy
                    hi = lo + rows
                    k = lo // 128
                    while lo < hi:
                        stop = min(hi, 128 * (k + 1))
                        out.append((k, lo - 128 * k, lo - a0 - 48 * dy,
                                    stop - lo))
                        lo = stop
                        k += 1
                    return out
                # DVE requires equal base partitions for SBUF operands, so
                # the +48/+96 partition-phase terms are staged through DMA
                # (which has no partition constraints), then added aligned.
                # Whole post-matmul chain in bf16: all-2-byte operands give
                # the DVE ADD/MULT their 2x mode, and the rounding errors
                # are damped by the rk scale (~0.07), staying below THETA.
                # The s48+s96 add runs on the DVE for the first two tiles
                # (shorter critical prefix) and on GpSimd afterwards
                # (offload, DVE saturated by then).
                s48 = accp.tile([128, L], mybir.dt.bfloat16, tag="s48")
                s96 = accp.tile([128, L], mybir.dt.bfloat16, tag="s96")
                acc = accp.tile([128, L], mybir.dt.bfloat16, tag="acc")
                accs = accp.tile([128, L], mybir.dt.bfloat16, tag="accs")
                for (k, srow, arow, n) in pieces(1):
                    nc.sync.dma_start(
                        s48[arow:arow + n, :],
                        sp_tiles[k][srow:srow + n, 48:48 + L])
                for (k, srow, arow, n) in pieces(2):
                    nc.sync.dma_start(
                        s96[arow:arow + n, :],
                        sp_tiles[k][srow:srow + n, 96:96 + L])
                # First and last tiles: adds on the DVE (shortest chain at
                # the pipeline ends); middle tiles: the s48+s96 add goes to
                # GpSimd (offload) but the +term0 add stays on the DVE —
                # two serial GpSimd passes per tile would outpace the DVE
                # and become the mid-pipeline limiter.
                eng = nc.vector if t == 0 else nc.gpsimd
                eng.tensor_add(acc[:rows, :], s48[:rows, :], s96[:rows, :])
                nc.vector.tensor_add(acc[:rows, :], acc[:rows, :],
                                     sp_tiles[t][:rows, 0:L])
                nc.vector.tensor_mul(accs[:rows, :], acc[:rows, :],
                                     rk_t[:rows, :])
                nc.vector.max(mxc[:rows, t, :], accs[:rows, :])
                nc.vector.max_index(mic[:rows, t, :], mxc[:rows, t, :],
                                    accs[:rows, :])

            def attn_tile_last():
                # The ragged 64-row last tile is repacked as [128, L/2]:
                # partitions 0:64 hold columns [0, L/2), partitions 64:128
                # hold columns [L/2, L) — every DVE pass runs at half width.
                # The staging DMAs do the partition/column remap for free;
                # the host merges the two 8-candidate halves.
                HW = L // 2
                t0h = accp.tile([128, HW], mybir.dt.bfloat16, tag="t0h")
                s48h = accp.tile([128, HW], mybir.dt.bfloat16, tag="s48h")
                s96h = accp.tile([128, HW], mybir.dt.bfloat16, tag="s96h")
                acch = accp.tile([128, HW], mybir.dt.bfloat16, tag="acch")
                accsh = accp.tile([128, HW], mybir.dt.bfloat16, tag="accsh")
                nc.sync.dma_start(t0h[0:64, :], sp_tiles[4][0:64, 0:HW])
                nc.sync.dma_start(t0h[64:128, :], sp_tiles[4][0:64, HW:L])
                nc.sync.dma_start(s48h[0:64, :],
                                  sp_tiles[4][48:112, 48:48 + HW])
                nc.sync.dma_start(s48h[64:128, :],
                                  sp_tiles[4][48:112, 48 + HW:48 + L])
                nc.sync.dma_start(s96h[0:32, :],
                                  sp_tiles[4][96:128, 96:96 + HW])
                nc.sync.dma_start(s96h[64:96, :],
                                  sp_tiles[4][96:128, 96 + HW:96 + L])
                nc.sync.dma_start(s96h[32:64, :],
                                  sp_tiles[5][0:32, 96:96 + HW])
                nc.sync.dma_start(s96h[96:128, :],
                                  sp_tiles[5][0:32, 96 + HW:96 + L])
                nc.vector.tensor_add(acch[:], t0h[:], s48h[:])
                nc.vector.tensor_add(acch[:], acch[:], s96h[:])
                nc.vector.tensor_mul(accsh[:], acch[:], rk2_t[:])
                nc.vector.max(mxc[:, 4, :], accsh[:])
                nc.vector.max_index(mic[:, 4, :], mxc[:, 4, :], accsh[:])

            make_sp(0)
            for t in range(1, n_sp):
                make_sp(t)
                if t - 1 < n_at - 1:
                    attn_tile(t - 1)
            attn_tile_last()
            nc.sync.dma_start(val_out[:], mxc[:])
            nc.sync.dma_start(idx_out[:], mic[:])

    nc.compile()
    return nc


def _host_prep(x1, x2, w):
    """Build qp/kp [B,600,UR] fp32, their padded bf16 device forms,
    rk fp32 [B,L] and rk64 [B,L]."""
    x1f = x1.transpose(0, 2, 3, 1, 4, 5).reshape(B, A, C, H, W_)
    x2f = x2.transpose(0, 2, 3, 1, 4, 5).reshape(B, A, C, H, W_)
    q = np.einsum('oc,bachw->baohw', w, x1f)   # [B, A, 8, H, W]
    k = np.einsum('oc,bachw->baohw', w, x2f)

    def chanshift(g):
        # g [B, A, 8, H, W] -> [B, 600, 50*48] with (a, c, dx) channels on a
        # vertically padded 50x48 grid
        gp = np.pad(g, ((0, 0), (0, 0), (0, 0), (0, 0), (1, 1)))
        sh = np.stack([gp[..., dx:dx + W_] for dx in range(3)], axis=3)
        sh = sh.reshape(B, CH, H, W_)
        sh = np.pad(sh, ((0, 0), (0, 0), (1, 1), (0, 0)))
        return np.ascontiguousarray(sh.reshape(B, CH, UR), dtype=np.float32)

    qp = chanshift(q)
    kp = chanshift(k)
    # rk[m] = 1 / ||K_m||, from padded per-pixel energy box-sums (fp64)
    ek = (k.astype(np.float64) ** 2).sum(axis=(1, 2))        # [B, H, W]
    ekp = np.pad(ek, ((0, 0), (1, 1), (1, 1)))
    kn = sum(ekp[:, dy:dy + H, dx:dx + W_]
             for dy in range(3) for dx in range(3))
    rk64 = (1.0 / np.maximum(np.sqrt(kn), 1e-12)).reshape(B, L)

    def to_dev(g):
        gb = g.astype(ml_dtypes.bfloat16)
        pad = np.zeros((B, CHP - CH, UR), ml_dtypes.bfloat16)
        return np.concatenate([gb, pad], axis=1).reshape(B, 5, 128, UR)

    return (qp, kp, to_dev(qp), to_dev(kp),
            rk64.astype(ml_dtypes.bfloat16), rk64)


def _resolve_idx(qp, kp, rk64, top8, flags):
    """Pick the exact (fp64) argmax among device candidates; rows with
    uncertifiably small top-8 spread get a full-row recompute."""
    nc_ = top8.shape[-1]
    idx = np.zeros((B, L), np.int64)
    for b in range(B):
        cand = top8[b].astype(np.int64)          # [L, nc_]
        q64 = qp[b].astype(np.float64)           # [600, UR]
        k64 = kp[b].astype(np.float64)
        score = np.zeros((L, nc_))
        for dy in (0, 48, 96):
            Qd = q64[:, dy:dy + L]               # [600, L]
            for c0 in range(0, L, 384):
                sl = slice(c0, c0 + 384)
                Kd = k64[:, cand[sl] + dy]       # [600, chunk, nc_]
                score[sl] += np.einsum('cr,crk->rk', Qd[:, sl], Kd)
        score *= rk64[b][cand]
        pick = np.argmax(score, axis=1)
        idx[b] = cand[np.arange(L), pick]

        flagged = np.where(flags[b])[0]
        if len(flagged):
            Qr = np.stack([q64[:, flagged + dy] for dy in (0, 48, 96)])
            Sr = np.einsum('dcr,cv->drv', Qr, k64)   # [3, R, UR]
            accs = (Sr[0][:, 0:L] + Sr[1][:, 48:48 + L]
                    + Sr[2][:, 96:96 + L]) * rk64[b][None, :]
            idx[b][flagged] = np.argmax(accs, axis=1)
    return idx


def _gather_fold(x3, idx):
    """Host epilogue: gather unfold(v) rows by idx and fold back."""
    v = x3.transpose(0, 2, 3, 1, 4, 5).reshape(B * A, C, H, W_)
    vp = np.pad(v, ((0, 0), (0, 0), (1, 1), (1, 1)))
    cols = np.stack([vp[:, :, i:i + H, j:j + W_]
                     for i in range(3) for j in range(3)], axis=2)
    V = cols.reshape(B, A, C * 9, L).transpose(0, 3, 1, 2).reshape(B, L, -1)
    outc = np.take_along_axis(V, idx[:, :, None], axis=1)
    p_v = C * 9
    outc = outc.reshape(B, L, A, p_v).transpose(0, 2, 3, 1)
    outc = outc.reshape(B * A, C, 3, 3, H, W_)
    out = np.zeros((B * A, C, H + 2, W_ + 2), np.float32)
    for i in range(3):
        for j in range(3):
            out[:, :, i:i + H, j:j + W_] += outc[:, :, i, j]
    out = out[:, :, 1:1 + H, 1:1 + W_]
    return np.ascontiguousarray(
        out.reshape(B, AH, AW, C, H, W_).transpose(0, 3, 1, 2, 4, 5))


def _make_in_maps(qpb, kpb, rk):
    in_maps = []
    for core in range(NCORES):
        b, r = core // 4, core % 4
        u0 = SLAB * r
        in_maps.append({
            "qpT": np.ascontiguousarray(qpb[b][:, :, u0:u0 + USLAB]),
            "kpT": kpb[b],
            "rk": np.broadcast_to(rk[b], (128, L)).copy(),
        })
    return in_maps


def kernel(x1, x2, x3, W):
    global _PROG
    sys.path.insert(0, '/opt/trn_rl_repo')
    from concourse.bass_utils import run_bass_kernel_spmd

    x1 = np.asarray(x1, dtype=np.float32)
    x2 = np.asarray(x2, dtype=np.float32)
    x3 = np.asarray(x3, dtype=np.float32)
    w = np.asarray(W, dtype=np.float32)

    qp, kp, qpb, kpb, rk, rk64 = _host_prep(x1, x2, w)
    in_maps = _make_in_maps(qpb, kpb, rk)

    if _PROG is None:
        _PROG = _build_program()
    res = run_bass_kernel_spmd(_PROG, in_maps, list(range(NCORES)))

    top8 = np.zeros((B, L, 16), np.int64)
    flags = np.zeros((B, L), bool)
    for core in range(NCORES):
        b, r = core // 4, core % 4
        base = SLAB * r
        ri = res.results[core]["idx"].astype(np.int64)      # [128, 5, 8]
        rv = res.results[core]["val"].astype(np.float32)
        mi = ri[:, 0:4, :].transpose(1, 0, 2).reshape(512, 8)
        mv = rv[:, 0:4, :].transpose(1, 0, 2).reshape(512, 8)
        top8[b, base:base + 512, 0:8] = mi
        top8[b, base:base + 512, 8:16] = mi
        flags[b, base:base + 512] = (mv[:, 0] - mv[:, 7]) < THETA
        # repacked last tile: two column halves per row
        h0i, h1i = ri[0:64, 4, :], ri[64:128, 4, :] + L // 2
        h0v, h1v = rv[0:64, 4, :], rv[64:128, 4, :]
        top8[b, base + 512:base + SLAB, 0:8] = h0i
        top8[b, base + 512:base + SLAB, 8:16] = h1i
        flags[b, base + 512:base + SLAB] = (
            (h0v[:, 0] - h0v[:, 7] < THETA)
            | (h1v[:, 0] - h1v[:, 7] < THETA))

    idx = _resolve_idx(qp, kp, rk64, top8, flags)
    return _gather_fold(x3, idx)


# revision 37
# speedup vs baseline: 1.1334x; 1.0400x over previous
"""Trainium2 Bass kernel for nn_CrossAttFA (retrieval_knn).

Math (reference):
  q = W @ x1 (1x1 conv, per-view), k = W @ x2, v = x3
  Q = l2norm(unfold3x3(q) regrouped to [b, L, 1800]), K likewise
  attn = Q @ K^T  [b, L, L];  idx = argmax(attn, -1)
  out = fold3x3(gather rows of unfold(v) by idx)

Device formulation (per batch b): fold the horizontal patch shift dx into
channels: qp[(a,c,dx), u] = q[a,c, uy-1, x+dx-1] over a vertically padded
50x48 pixel grid (u = uy*48+x, uy in [0,50)).  Then with
  S[u, v] = sum_ch qp[ch, u] * kp[ch, v]           (600-dim contraction)
  attn[n, m] = sum_{dy in 0..2} S[n + 48*dy, m + 48*dy]
argmax_m attn[n,m]/||K_m|| equals the reference argmax (column scale rk[m]
preserved; row scale 1/||Q_n|| does not affect argmax).

Precision scheme: the device computes attn in bf16 (inputs rounded to
bf16; matmul products exact, fp32 accumulate) and returns the TOP-8
values + indices per row (hardware max/max_index emit 8 lanes anyway).
The host rescores the <=8 candidates exactly in fp64 and picks the true
argmax; rows whose device top-8 spread is too small to certify coverage
(v1 - v8 < THETA) get a full exact row recompute.  Measured device error
is <5e-3 abs while the min top-8 spread is 4e-2, so flags are rare.

Sharding: 8 cores = 2 batches x 4 row-slabs of 576 attention rows each.
Each core computes S rows for its slab (+96 halo), the 3-term diagonal
box-sum (one add on GpSimd, one on DVE), the rk column scale, and the
hardware top-8 max/max_index.  Host does the 1x1 conv + layout prep and
the rescore + gather/fold epilogue.
"""
import sys

sys.path.insert(0, '/opt/trn_rl_repo')
import numpy as np
import ml_dtypes

B, C, AH, AW, H, W_ = 2, 64, 5, 5, 48, 48
A = AH * AW                  # 25 views
L = H * W_                   # 2304 pixels
CH = A * C // 8 * 3          # 600 channels (a, c_out=8, dx=3)
CO = 8                       # conv output channels
CHP = 640                    # padded to 5 K-chunks of 128
UR = 2400                    # padded u-grid rows (50 x 48)
NCORES = 8
SLAB = L // 4                # 576 attn rows per core
USLAB = SLAB + 96            # S rows needed per core (incl. +48,+96 halo)
NT = 480                     # matmul moving free dim (psum bank = 512 fp32)
NB = 480                     # matmul moving free dim (ISA caps at 512)
THETA = 0.022                # top-8 spread flag threshold (abs, scaled units)
KLAST = 88                   # real channels in the 5th K-chunk (600 - 512)

_PROG = None


def _build_program():
    import concourse.bass as bass
    import concourse.bacc as bacc
    import concourse.mybir as mybir
    from concourse.tile import TileContext

    nc = bacc.Bacc('TRN2', target_bir_lowering=False, debug=False,
                   num_devices=NCORES)
    qpT_in = nc.declare_dram_parameter("qpT", [5, 128, USLAB],
                                       mybir.dt.bfloat16, isOutput=False)
    kpT_in = nc.declare_dram_parameter("kpT", [5, 128, UR],
                                       mybir.dt.bfloat16, isOutput=False)
    rk_in = nc.declare_dram_parameter("rk", [128, L],
                                      mybir.dt.bfloat16, isOutput=False)
    idx_out = nc.declare_dram_parameter("idx", [128, 5, 8],
                                        mybir.dt.uint16, isOutput=True)
    val_out = nc.declare_dram_parameter("val", [128, 5, 8],
                                        mybir.dt.bfloat16, isOutput=True)

    n_sp = (USLAB + 127) // 128          # 6 S-row tiles (last is 32 rows)
    sp_rows = [min(128, USLAB - 128 * t) for t in range(n_sp)]
    n_at = (SLAB + 127) // 128           # 5 attn tiles (last is 64 rows)
    at_rows = [min(128, SLAB - 128 * t) for t in range(n_at)]

    with TileContext(nc) as tc:
        with tc.tile_pool(name="inp", bufs=1) as inp, \
             tc.tile_pool(name="sp", bufs=6) as spp, \
             tc.tile_pool(name="acc", bufs=2) as accp, \
             tc.tile_pool(name="res", bufs=2) as resp, \
             tc.tile_pool(name="ps", bufs=1, space="PSUM") as psp:

            kp_t = [inp.tile([128, UR], mybir.dt.bfloat16, tag=f"kp{i}",
                             name=f"kp{i}") for i in range(5)]
            qp_t = [inp.tile([128, USLAB], mybir.dt.bfloat16, tag=f"qp{i}",
                             name=f"qp{i}") for i in range(5)]
            rk_t = inp.tile([128, L], mybir.dt.bfloat16, tag="rk")
            rk2_t = inp.tile([128, L // 2], mybir.dt.bfloat16, tag="rk2")
            # One DMA per kc chunk: more SP issues would serialize on the
            # SP sequencer (~0.7us each) and delay the staging DMAs that
            # feed the DVE.  qp/rk go on the Act queue.
            # All input issues on SP: the Act queue must stay free for the
            # psum->sbuf copies (its FIFO head otherwise delays them ~8us).
            for i in range(5):
                rows_k = KLAST if i == 4 else 128
                nc.sync.dma_start(kp_t[i][:rows_k, :], kpT_in[i][:rows_k, :])
                nc.sync.dma_start(qp_t[i][:rows_k, :], qpT_in[i][:rows_k, :])
            nc.sync.dma_start(rk_t[:], rk_in[:])
            # rk2: per-half rk for the repacked last attn tile
            nc.sync.dma_start(rk2_t[0:64, :], rk_t[0:64, 0:L // 2])
            nc.sync.dma_start(rk2_t[64:128, :], rk_t[0:64, L // 2:L])
            mxc = inp.tile([128, 5, 8], mybir.dt.bfloat16, tag="mxc")
            mic = inp.tile([128, 5, 8], mybir.dt.uint16, tag="mic")
            # GpSimd ucode warm-up: its first tensor_add pays a
            # LIBRARY_RELOAD; absorb it at t=0 instead of mid-pipeline.
            warm = inp.tile([128, 8], mybir.dt.bfloat16, tag="warm")
            nc.gpsimd.memset(warm[:], 0)
            nc.gpsimd.tensor_add(warm[:], warm[:], warm[:])

            sp_tiles = [None] * n_sp

            def make_sp(t):
                # j-outer loop: each PSUM bank finishes its 5-chunk
                # accumulation then is copied out while the next bank
                # computes, so bank j is free again well before the next
                # S tile needs it (no PSUM stall with bufs=1).  S sbuf
                # copy is bf16 (halves staging DMA bytes; the rounding is
                # scaled down by rk and stays well inside the margin).
                rows = sp_rows[t]
                sp = spp.tile([128, UR], mybir.dt.bfloat16, tag="sp")
                for j, c0 in enumerate(range(0, UR, NB)):
                    w = min(NB, UR - c0)
                    ps = psp.tile([128, w], mybir.dt.float32, tag=f"ps{j}",
                                  name=f"ps{j}")
                    for kc in range(5):
                        kk = KLAST if kc == 4 else 128
                        nc.tensor.matmul(
                            ps[:rows, :],
                            qp_t[kc][:kk, 128 * t:128 * t + rows],
                            kp_t[kc][:kk, c0:c0 + w],
                            start=(kc == 0), stop=(kc == 4))
                    # Copies of the first two S tiles run on the DVE (idle
                    # until attn(0) anyway) — the Act copy stream otherwise
                    # starts ~8us late and gates the whole attn pipeline.
                    if t < 2:
                        nc.vector.tensor_copy(sp[:rows, c0:c0 + w],
                                              ps[:rows, :])
                    else:
                        nc.scalar.copy(sp[:rows, c0:c0 + w], ps[:rows, :])
                sp_tiles[t] = sp

            def attn_tile(t):
                rows = at_rows[t]
                a0 = 128 * t  # slab-local first attn row of this tile
                # term dy contributes S[a0+r+48dy, m+48dy]; S tile k holds
                # rows [128k, 128k + sp_rows[k]).
                def pieces(dy):
                    out = []
                    lo = a0 + 48 * dy
                    hi = lo + rows
                    k = lo // 128
                    while lo < hi:
                        stop = min(hi, 128 * (k + 1))
                        out.append((k, lo - 128 * k, lo - a0 - 48 * dy,
                                    stop - lo))
                        lo = stop
                        k += 1
                    return out
                # DVE requires equal base partitions for SBUF operands, so
                # the +48/+96 partition-phase terms are staged through DMA
                # (which has no partition constraints), then added aligned.
                # Whole post-matmul chain in bf16: all-2-byte operands give
                # the DVE ADD/MULT their 2x mode, and the rounding errors
                # are damped by the rk scale (~0.07), staying below THETA.
                # The s48+s96 add runs on the DVE for the first two tiles
                # (shorter critical prefix) and on GpSimd afterwards
                # (offload, DVE saturated by then).
                s48 = accp.tile([128, L], mybir.dt.bfloat16, tag="s48")
                s96 = accp.tile([128, L], mybir.dt.bfloat16, tag="s96")
                acc = accp.tile([128, L], mybir.dt.bfloat16, tag="acc")
                accs = accp.tile([128, L], mybir.dt.bfloat16, tag="accs")
                for (k, srow, arow, n) in pieces(1):
                    nc.sync.dma_start(
                        s48[arow:arow + n, :],
                        sp_tiles[k][srow:srow + n, 48:48 + L])
                for (k, srow, arow, n) in pieces(2):
                    nc.sync.dma_start(
                        s96[arow:arow + n, :],
                        sp_tiles[k][srow:srow + n, 96:96 + L])
                # First and last tiles: adds on the DVE (shortest chain at
                # the pipeline ends); middle tiles: the s48+s96 add goes to
                # GpSimd (offload) but the +term0 add stays on the DVE —
                # two serial GpSimd passes per tile would outpace the DVE
                # and become the mid-pipeline limiter.
                eng = nc.vector if t <= 1 else nc.gpsimd
                eng.tensor_add(acc[:rows, :], s48[:rows, :], s96[:rows, :])
                nc.vector.tensor_add(acc[:rows, :], acc[:rows, :],
                                     sp_tiles[t][:rows, 0:L])
                nc.vector.tensor_mul(accs[:rows, :], acc[:rows, :],
                                     rk_t[:rows, :])
                nc.vector.max(mxc[:rows, t, :], accs[:rows, :])
                nc.vector.max_index(mic[:rows, t, :], mxc[:rows, t, :],
                                    accs[:rows, :])

            def attn_tile_last():
                # The ragged 64-row last tile is repacked as [128, L/2]:
                # partitions 0:64 hold columns [0, L/2), partitions 64:128
                # hold columns [L/2, L) — every DVE pass runs at half width.
                # The staging DMAs do the partition/column remap for free;
                # the host merges the two 8-candidate halves.
                HW = L // 2
                t0h = accp.tile([128, HW], mybir.dt.bfloat16, tag="t0h")
                s48h = accp.tile([128, HW], mybir.dt.bfloat16, tag="s48h")
                s96h = accp.tile([128, HW], mybir.dt.bfloat16, tag="s96h")
                acch = accp.tile([128, HW], mybir.dt.bfloat16, tag="acch")
                accsh = accp.tile([128, HW], mybir.dt.bfloat16, tag="accsh")
                nc.sync.dma_start(t0h[0:64, :], sp_tiles[4][0:64, 0:HW])
                nc.sync.dma_start(t0h[64:128, :], sp_tiles[4][0:64, HW:L])
                nc.sync.dma_start(s48h[0:64, :],
                                  sp_tiles[4][48:112, 48:48 + HW])
                nc.sync.dma_start(s48h[64:128, :],
                                  sp_tiles[4][48:112, 48 + HW:48 + L])
                nc.sync.dma_start(s96h[0:32, :],
                                  sp_tiles[4][96:128, 96:96 + HW])
                nc.sync.dma_start(s96h[64:96, :],
                                  sp_tiles[4][96:128, 96 + HW:96 + L])
                nc.sync.dma_start(s96h[32:64, :],
                                  sp_tiles[5][0:32, 96:96 + HW])
                nc.sync.dma_start(s96h[96:128, :],
                                  sp_tiles[5][0:32, 96 + HW:96 + L])
                nc.vector.tensor_add(acch[:], t0h[:], s48h[:])
                nc.vector.tensor_add(acch[:], acch[:], s96h[:])
                nc.vector.tensor_mul(accsh[:], acch[:], rk2_t[:])
                nc.vector.max(mxc[:, 4, :], accsh[:])
                nc.vector.max_index(mic[:, 4, :], mxc[:, 4, :], accsh[:])

            make_sp(0)
            for t in range(1, n_sp):
                make_sp(t)
                if t - 1 < n_at - 1:
                    attn_tile(t - 1)
            attn_tile_last()
            nc.sync.dma_start(val_out[:], mxc[:])
            nc.sync.dma_start(idx_out[:], mic[:])

    nc.compile()
    return nc


def _host_prep(x1, x2, w):
    """Build qp/kp [B,600,UR] fp32, their padded bf16 device forms,
    rk fp32 [B,L] and rk64 [B,L]."""
    x1f = x1.transpose(0, 2, 3, 1, 4, 5).reshape(B, A, C, H, W_)
    x2f = x2.transpose(0, 2, 3, 1, 4, 5).reshape(B, A, C, H, W_)
    q = np.einsum('oc,bachw->baohw', w, x1f)   # [B, A, 8, H, W]
    k = np.einsum('oc,bachw->baohw', w, x2f)

    def chanshift(g):
        # g [B, A, 8, H, W] -> [B, 600, 50*48] with (a, c, dx) channels on a
        # vertically padded 50x48 grid
        gp = np.pad(g, ((0, 0), (0, 0), (0, 0), (0, 0), (1, 1)))
        sh = np.stack([gp[..., dx:dx + W_] for dx in range(3)], axis=3)
        sh = sh.reshape(B, CH, H, W_)
        sh = np.pad(sh, ((0, 0), (0, 0), (1, 1), (0, 0)))
        return np.ascontiguousarray(sh.reshape(B, CH, UR), dtype=np.float32)

    qp = chanshift(q)
    kp = chanshift(k)
    # rk[m] = 1 / ||K_m||, from padded per-pixel energy box-sums (fp64)
    ek = (k.astype(np.float64) ** 2).sum(axis=(1, 2))        # [B, H, W]
    ekp = np.pad(ek, ((0, 0), (1, 1), (1, 1)))
    kn = sum(ekp[:, dy:dy + H, dx:dx + W_]
             for dy in range(3) for dx in range(3))
    rk64 = (1.0 / np.maximum(np.sqrt(kn), 1e-12)).reshape(B, L)

    def to_dev(g):
        gb = g.astype(ml_dtypes.bfloat16)
        pad = np.zeros((B, CHP - CH, UR), ml_dtypes.bfloat16)
        return np.concatenate([gb, pad], axis=1).reshape(B, 5, 128, UR)

    return (qp, kp, to_dev(qp), to_dev(kp),
            rk64.astype(ml_dtypes.bfloat16), rk64)


def _resolve_idx(qp, kp, rk64, top8, flags):
    """Pick the exact (fp64) argmax among device candidates; rows with
    uncertifiably small top-8 spread get a full-row recompute."""
    nc_ = top8.shape[-1]
    idx = np.zeros((B, L), np.int64)
    for b in range(B):
        cand = top8[b].astype(np.int64)          # [L, nc_]
        q64 = qp[b].astype(np.float64)           # [600, UR]
        k64 = kp[b].astype(np.float64)
        score = np.zeros((L, nc_))
        for dy in (0, 48, 96):
            Qd = q64[:, dy:dy + L]               # [600, L]
            for c0 in range(0, L, 384):
                sl = slice(c0, c0 + 384)
                Kd = k64[:, cand[sl] + dy]       # [600, chunk, nc_]
                score[sl] += np.einsum('cr,crk->rk', Qd[:, sl], Kd)
        score *= rk64[b][cand]
        pick = np.argmax(score, axis=1)
        idx[b] = cand[np.arange(L), pick]

        flagged = np.where(flags[b])[0]
        if len(flagged):
            Qr = np.stack([q64[:, flagged + dy] for dy in (0, 48, 96)])
            Sr = np.einsum('dcr,cv->drv', Qr, k64)   # [3, R, UR]
            accs = (Sr[0][:, 0:L] + Sr[1][:, 48:48 + L]
                    + Sr[2][:, 96:96 + L]) * rk64[b][None, :]
            idx[b][flagged] = np.argmax(accs, axis=1)
    return idx


def _gather_fold(x3, idx):
    """Host epilogue: gather unfold(v) rows by idx and fold back."""
    v = x3.transpose(0, 2, 3, 1, 4, 5).reshape(B * A, C, H, W_)
    vp = np.pad(v, ((0, 0), (0, 0), (1, 1), (1, 1)))
    cols = np.stack([vp[:, :, i:i + H, j:j + W_]
                     for i in range(3) for j in range(3)], axis=2)
    V = cols.reshape(B, A, C * 9, L).transpose(0, 3, 1, 2).reshape(B, L, -1)
    outc = np.take_along_axis(V, idx[:, :, None], axis=1)
    p_v = C * 9
    outc = outc.reshape(B, L, A, p_v).transpose(0, 2, 3, 1)
    outc = outc.reshape(B * A, C, 3, 3, H, W_)
    out = np.zeros((B * A, C, H + 2, W_ + 2), np.float32)
    for i in range(3):
        for j in range(3):
            out[:, :, i:i + H, j:j + W_] += outc[:, :, i, j]
    out = out[:, :, 1:1 + H, 1:1 + W_]
    return np.ascontiguousarray(
        out.reshape(B, AH, AW, C, H, W_).transpose(0, 3, 1, 2, 4, 5))


def _make_in_maps(qpb, kpb, rk):
    in_maps = []
    for core in range(NCORES):
        b, r = core // 4, core % 4
        u0 = SLAB * r
        in_maps.append({
            "qpT": np.ascontiguousarray(qpb[b][:, :, u0:u0 + USLAB]),
            "kpT": kpb[b],
            "rk": np.broadcast_to(rk[b], (128, L)).copy(),
        })
    return in_maps


def kernel(x1, x2, x3, W):
    global _PROG
    sys.path.insert(0, '/opt/trn_rl_repo')
    from concourse.bass_utils import run_bass_kernel_spmd

    x1 = np.asarray(x1, dtype=np.float32)
    x2 = np.asarray(x2, dtype=np.float32)
    x3 = np.asarray(x3, dtype=np.float32)
    w = np.asarray(W, dtype=np.float32)

    qp, kp, qpb, kpb, rk, rk64 = _host_prep(x1, x2, w)
    in_maps = _make_in_maps(qpb, kpb, rk)

    if _PROG is None:
        _PROG = _build_program()
    res = run_bass_kernel_spmd(_PROG, in_maps, list(range(NCORES)))

    top8 = np.zeros((B, L, 16), np.int64)
    flags = np.zeros((B, L), bool)
    for core in range(NCORES):
        b, r = core // 4, core % 4
        base = SLAB * r
        ri = res.results[core]["idx"].astype(np.int64)      # [128, 5, 8]
        rv = res.results[core]["val"].astype(np.float32)
        mi = ri[:, 0:4, :].transpose(1, 0, 2).reshape(512, 8)
        mv = rv[:, 0:4, :].transpose(1, 0, 2).reshape(512, 8)
        top8[b, base:base + 512, 0:8] = mi
        top8[b, base:base + 512, 8:16] = mi
        flags[b, base:base + 512] = (mv[:, 0] - mv[:, 7]) < THETA
        # repacked last tile: two column halves per row
        h0i, h1i = ri[0:64, 4, :], ri[64:128, 4, :] + L // 2
        h0v, h1v = rv[0:64, 4, :], rv[64:128, 4, :]
        top8[b, base + 512:base + SLAB, 0:8] = h0i
        top8[b, base + 512:base + SLAB, 8:16] = h1i
        flags[b, base + 512:base + SLAB] = (
            (h0v[:, 0] - h0v[:, 7] < THETA)
            | (h1v[:, 0] - h1v[:, 7] < THETA))

    idx = _resolve_idx(qp, kp, rk64, top8, flags)
    return _gather_fold(x3, idx)
